# revision 6
# baseline (speedup 1.0000x reference)
"""Trainium2 Bass kernel for nn_NeuralALU (batched byte-encoded 32-bit add).

The reference network computes, per batch element, a chain of table-lookup
matmuls + sharp softmaxes (scale=100) over exactly-one-hot byte encodings.
Because the inputs are exact one-hots, the float pipeline collapses to a
discrete algorithm (validated to ~1e-22 rel-err):

  s = a + b                      per 256-wide byte block (<=2 nonzeros)
  z = dot(s, code2)              code2 packs lo/hi nibble sums of 2 bytes
                                 into four 6-bit fields (exact in f32)
  xnib[n] in [0,30]              per-nibble sums, carry order lo0,hi0,...
  soft carry chain:  Y_n = x_n + c_n,  c_{n+1} = clamp(Y_n - 15, 0, 1)
                     (kept shifted: c~ = c + 15 so both steps are 1 op)
  soft dist:  ymod = Y - 16*[Y >= 15.75]
              s17[j] = relu(1 - |ymod - j|),  j = 0..16   (17-wide)
              dist[k] = s17[k],  dist[0] += s17[16]       (wraparound)
       This one triangle kernel reproduces onehot(U)*(1-P/2) +
       onehot((U+1)%16)*(P/2) for carry states c in {0, 0.5, 1}.
  out byte row [256] = outer(h_dist, l_dist) flattened

Engine plan (per core, pure data-parallel over batch):
  - DVE: s=a+b, packed dots, field extraction, carry chain, dist prep,
    plus the last chunk's outer products (tail shortening).
  - Pool (gpsimd): outer products of the non-final chunks.
  - ACT (scalar): |.| and relu of the soft dists; issues output stores
    on its own HWDGE ring so loads and stores never share a FIFO.
  - 3 chunks (12/12/8 tiles) so the output stream starts early and the
    post-input tail only contains the last small chunk.
"""

import numpy as np

import concourse.bass as bass
import concourse.bacc as bacc
import concourse.mybir as mybir
from concourse.tile import TileContext
from concourse.bass_utils import run_bass_kernel_spmd

N_CORES = 8
B_FULL = 32768
ROWS = B_FULL // N_CORES  # 4096 rows per core
F = 1024  # 4 bytes x 256 one-hot
P = 128
CHUNKS = (12, 12, 8)            # tiles per chunk
BATCHES = ((2, 2, 4, 4), (4, 4, 4), (4, 4))  # input batch plan per chunk
MAXC = max(CHUNKS)
BMAX = 4

FP = mybir.dt.float32
I32 = mybir.dt.int32


def _const_tables():
    k = np.arange(256)
    nib = ((k % 16) + 64.0 * (k // 16)).astype(np.float32)  # <= 975
    # two bytes per dot: second byte's fields scaled by 4096 (sums stay
    # exact in f32: max 2*975*4096 + 2*975 < 2^23)
    code2 = np.concatenate([nib, nib * 4096.0])  # [512]
    code2 = np.broadcast_to(code2, (P, 512)).copy()
    iota17 = np.broadcast_to(np.arange(17, dtype=np.float32), (P, 17)).copy()
    return code2, iota17


def build_nc(rows=ROWS):
    nt = rows // P
    assert sum(CHUNKS) == nt

    AL = mybir.AluOpType
    nc = bacc.Bacc()
    a_d = nc.declare_dram_parameter("a", [rows, F], FP, isOutput=False)
    b_d = nc.declare_dram_parameter("b", [rows, F], FP, isOutput=False)
    code_d = nc.declare_dram_parameter("code2", [P, 512], FP, isOutput=False)
    iota_d = nc.declare_dram_parameter("iota17", [P, 17], FP, isOutput=False)
    out_d = nc.declare_dram_parameter("out", [rows, F], FP, isOutput=True)

    def dram_view(d, g0, ntiles):  # [p, t, f] view of tiles [g0, g0+ntiles)
        return d[g0 * P : (g0 + ntiles) * P, :].rearrange(
            "(t p) f -> p t f", t=ntiles, p=P
        )

    with TileContext(nc) as tc:
        with (
            tc.tile_pool(name="consts", bufs=1) as cpool,
            tc.tile_pool(name="io", bufs=3) as iopool,
            tc.tile_pool(name="sums", bufs=2) as spool,
            tc.tile_pool(name="scratch", bufs=2) as scpool,
            tc.tile_pool(name="arrs", bufs=2) as apool,
            tc.tile_pool(name="outp", bufs=2) as opool,
        ):
            code_raw = cpool.tile([P, 512], FP, tag="code_raw")
            code2 = cpool.tile([P, 512], FP, tag="code2")
            iota_raw = cpool.tile([P, 17], FP, tag="iota_raw")
            iota17 = cpool.tile([P, 17], FP, tag="iota17")
            nc.sync.dma_start(code_raw[:, :], code_d[:, :])
            nc.sync.dma_start(iota_raw[:, :], iota_d[:, :])
            # pre-touch consts on DVE so compute ops only wait on DVE state
            nc.vector.tensor_copy(code2[:, :], code_raw[:, :])
            nc.vector.tensor_copy(iota17[:, :], iota_raw[:, :])

            ch_base = 0
            for ch, ntc in enumerate(CHUNKS):
                last_ch = ch == len(CHUNKS) - 1
                z2 = apool.tile([P, 2 * MAXC], FP, tag="z2")
                z2_i = apool.tile([P, 2 * MAXC], I32, tag="z2i")
                xnib_i = apool.tile([P, 8 * MAXC], I32, tag="xnibi")
                xnib = apool.tile([P, 8 * MAXC], FP, tag="xnib")
                chist = apool.tile([P, 9 * MAXC], FP, tag="chist")
                y_all = apool.tile([P, 8 * MAXC], FP, tag="yall")
                wrap = apool.tile([P, 8 * MAXC], FP, tag="wrap")
                ymod = apool.tile([P, 8 * MAXC], FP, tag="ymod")
                dtile = apool.tile([P, 8 * MAXC * 17], FP, tag="dtile")
                s17 = apool.tile([P, 8 * MAXC * 17], FP, tag="s17")

                # ---- phase 1: batched loads, s = a+b, packed dots ----
                t = 0
                for bsz in BATCHES[ch]:
                    g0 = ch_base + t
                    a_b = iopool.tile([P, BMAX * F], FP, tag="ab")
                    b_b = iopool.tile([P, BMAX * F], FP, tag="bb")
                    s_b = spool.tile([P, BMAX * F], FP, tag="sb")
                    nc.sync.dma_start(
                        a_b[:, : bsz * F].rearrange("p (t f) -> p t f", t=bsz),
                        dram_view(a_d, g0, bsz),
                    )
                    nc.sync.dma_start(
                        b_b[:, : bsz * F].rearrange("p (t f) -> p t f", t=bsz),
                        dram_view(b_d, g0, bsz),
                    )
                    for ti in range(bsz):
                        sl = slice(ti * F, (ti + 1) * F)
                        nc.vector.tensor_add(s_b[:, sl], a_b[:, sl], b_b[:, sl])
                        for i2 in range(2):
                            prod = scpool.tile([P, 512], FP, tag="prod")
                            nc.vector.scalar_tensor_tensor(
                                out=prod[:, :],
                                in0=s_b[:, ti * F + i2 * 512 : ti * F + (i2 + 1) * 512],
                                scalar=1.0,
                                in1=code2[:, :],
                                op0=AL.mult,
                                op1=AL.mult,
                                accum_out=z2[:, i2 * ntc + t + ti : i2 * ntc + t + ti + 1],
                            )
                    t += bsz

                # ---- phase 2: unpack four 6-bit fields -> xnib ----
                nc.vector.tensor_copy(z2_i[:, : 2 * ntc], z2[:, : 2 * ntc])
                z2_v = z2_i[:, : 2 * ntc].rearrange("p (i2 t) -> p i2 t", t=ntc)
                xn_v = xnib_i[:, : 8 * ntc].rearrange(
                    "p (i2 k t) -> p i2 k t", i2=2, k=4
                )
                nc.vector.tensor_scalar(
                    out=xn_v[:, :, 0, :], in0=z2_v, scalar1=63, scalar2=None,
                    op0=AL.bitwise_and,
                )
                nc.vector.tensor_scalar(
                    out=xn_v[:, :, 1, :], in0=z2_v, scalar1=6, scalar2=63,
                    op0=AL.logical_shift_right, op1=AL.bitwise_and,
                )
                nc.vector.tensor_scalar(
                    out=xn_v[:, :, 2, :], in0=z2_v, scalar1=12, scalar2=63,
                    op0=AL.logical_shift_right, op1=AL.bitwise_and,
                )
                nc.vector.tensor_scalar(
                    out=xn_v[:, :, 3, :], in0=z2_v, scalar1=18, scalar2=None,
                    op0=AL.logical_shift_right,
                )
                nc.vector.tensor_copy(xnib[:, : 8 * ntc], xnib_i[:, : 8 * ntc])

                # ---- phase 3: carry chain, shifted carry c~ = c + 15 ----
                nc.vector.memset(chist[:, 0:ntc], 15.5)
                for n in range(8):
                    y_n = y_all[:, n * ntc : (n + 1) * ntc]
                    nc.vector.scalar_tensor_tensor(
                        out=y_n, in0=xnib[:, n * ntc : (n + 1) * ntc],
                        scalar=-15.0, in1=chist[:, n * ntc : (n + 1) * ntc],
                        op0=AL.add, op1=AL.add,
                    )
                    nc.vector.tensor_scalar(
                        out=chist[:, (n + 1) * ntc : (n + 2) * ntc], in0=y_n,
                        scalar1=15.0, scalar2=16.0, op0=AL.max, op1=AL.min,
                    )

                # ---- phase 4: wrap to ymod in [0, 15.5] ----
                nc.vector.tensor_scalar(
                    out=wrap[:, : 8 * ntc], in0=y_all[:, : 8 * ntc],
                    scalar1=15.75, scalar2=None, op0=AL.is_ge,
                )
                nc.vector.scalar_tensor_tensor(
                    out=ymod[:, : 8 * ntc], in0=wrap[:, : 8 * ntc], scalar=-16.0,
                    in1=y_all[:, : 8 * ntc], op0=AL.mult, op1=AL.add,
                )

                # ---- phase 5: soft dists s17 = relu(1 - |ymod - j|) ----
                # |.| and relu run on the otherwise-idle ACT engine
                G = 8 * ntc
                d_v = dtile[:, : G * 17].rearrange("p (g k) -> p g k", k=17)
                s_v = s17[:, : G * 17].rearrange("p (g k) -> p g k", k=17)
                ymod_b = ymod[:, :G, None].broadcast_to([P, G, 17])
                iota_b = iota17[:, None, :].broadcast_to([P, G, 17])
                nc.vector.tensor_tensor(d_v, ymod_b, iota_b, op=AL.subtract)
                nc.scalar.activation(
                    dtile[:, : G * 17], dtile[:, : G * 17],
                    mybir.ActivationFunctionType.Abs,
                )
                nc.scalar.activation(
                    s17[:, : G * 17], dtile[:, : G * 17],
                    mybir.ActivationFunctionType.Relu, bias=1.0, scale=-1.0,
                )
                # wraparound fold: dist[0] += s17[16]
                nc.vector.tensor_add(
                    s_v[:, :, 0:1], s_v[:, :, 0:1], s_v[:, :, 16:17]
                )

                # ---- phase 6: outer products + stores ----
                s5 = s17[:, : G * 17].rearrange(
                    "p (i two t k) -> p i two t k", two=2, t=ntc, k=17
                )
                for t0 in range(0, ntc, 4):
                    o4 = opool.tile([P, 4 * F], FP, tag="o4")
                    for ti in range(4):
                        t = t0 + ti
                        o_vv = o4[:, ti * F : (ti + 1) * F].rearrange(
                            "p (i h l) -> p i h l", h=16, l=16
                        )
                        h_b = s5[:, :, 1, t, 0:16][:, :, :, None].broadcast_to(
                            [P, 4, 16, 16])
                        l_b = s5[:, :, 0, t, 0:16][:, :, None, :].broadcast_to(
                            [P, 4, 16, 16])
                        # last chunk: split outers DVE/Pool to halve the tail
                        eng = nc.vector if (last_ch and ti % 2 == 0) else nc.gpsimd
                        eng.tensor_mul(o_vv, h_b, l_b)
                    nc.scalar.dma_start(
                        dram_view(out_d, ch_base + t0, 4),
                        o4[:, :].rearrange("p (t f) -> p t f", t=4),
                    )
                ch_base += ntc

    nc.finalize()
    return nc


_NC_CACHE = {}
LAST_RESULT = None


def kernel(**inputs) -> np.ndarray:
    global LAST_RESULT
    a = np.ascontiguousarray(np.asarray(inputs["a"], dtype=np.float32)).reshape(B_FULL, F)
    b = np.ascontiguousarray(np.asarray(inputs["b"], dtype=np.float32)).reshape(B_FULL, F)
    code2, iota17 = _const_tables()

    if ROWS not in _NC_CACHE:
        _NC_CACHE[ROWS] = build_nc(ROWS)
    nc = _NC_CACHE[ROWS]

    in_maps = []
    for c in range(N_CORES):
        in_maps.append({
            "a": a[c * ROWS : (c + 1) * ROWS],
            "b": b[c * ROWS : (c + 1) * ROWS],
            "code2": code2,
            "iota17": iota17,
        })
    res = run_bass_kernel_spmd(nc, in_maps, core_ids=list(range(N_CORES)))
    LAST_RESULT = res
    out = np.concatenate([r["out"] for r in res.results], axis=0)
    return out.reshape(B_FULL, 4, 256)


# revision 11
# speedup vs baseline: 1.0283x; 1.0283x over previous
"""Trainium2 Bass kernel for nn_NeuralALU (batched byte-encoded 32-bit add).

The reference network computes, per batch element, a chain of table-lookup
matmuls + sharp softmaxes (scale=100) over exactly-one-hot byte encodings.
Because the inputs are exact one-hots, the float pipeline collapses to a
discrete algorithm (validated to ~1e-22 rel-err):

  s = a + b                      per 256-wide byte block (<=2 nonzeros)
  z = dot(s, code2)              code2 packs lo/hi nibble sums of 2 bytes
                                 into four 6-bit fields (exact in f32)
  xnib[n] in [0,30]              per-nibble sums, carry order lo0,hi0,...
  soft carry chain:  Y_n = x_n + c_n,  c_{n+1} = clamp(Y_n - 15, 0, 1)
                     (kept shifted: c~ = c + 15 so both steps are 1 op)
  soft dist:  ymod = Y - 16*[Y >= 15.75]
              s17[j] = relu(1 - |ymod - j|),  j = 0..16   (17-wide)
              dist[k] = s17[k],  dist[0] += s17[16]       (wraparound)
       This one triangle kernel reproduces onehot(U)*(1-P/2) +
       onehot((U+1)%16)*(P/2) for carry states c in {0, 0.5, 1}.
  out byte row [256] = outer(h_dist, l_dist) flattened

Engine plan (per core, pure data-parallel over batch):
  - DVE: s=a+b (reading both halves of one concat tile), packed dots,
    field extraction, carry chain, dist prep, and a contiguous block of
    the last chunk's outer products (tail shortening).
  - Pool (gpsimd): all other outer products.
  - ACT (scalar): |.| and relu of the soft dists; issues output stores
    on its own HWDGE ring so loads and stores never share a FIFO.
  - a and b are concatenated host-side so one DMA stream feeds phase 1;
    2-tile (2MB) input batches, deep-buffered, first batches smaller to
    cut pipeline lead-in.
"""

import numpy as np

import concourse.bass as bass
import concourse.bacc as bacc
import concourse.mybir as mybir
from concourse.tile import TileContext
from concourse.bass_utils import run_bass_kernel_spmd

N_CORES = 8
B_FULL = 32768
ROWS = B_FULL // N_CORES  # 4096 rows per core
F = 1024  # 4 bytes x 256 one-hot
P = 128
CHUNKS = (16, 16)
BATCHES = ((1, 1, 2, 2, 2, 2, 2, 2, 2), (2,) * 8)  # input batch plan
BMAX = 2
DVE_TAIL_OUTERS = 6  # last chunk: final N tiles' outers run on DVE

FP = mybir.dt.float32
BF = mybir.dt.bfloat16
I32 = mybir.dt.int32


def _const_tables():
    k = np.arange(256)
    nib = ((k % 16) + 64.0 * (k // 16)).astype(np.float32)  # <= 975
    # two bytes per dot: second byte's fields scaled by 4096 (sums stay
    # exact in f32: max 2*975*4096 + 2*975 < 2^23)
    code2 = np.concatenate([nib, nib * 4096.0])  # [512]
    code2 = np.broadcast_to(code2, (P, 512)).copy()
    iota17 = np.broadcast_to(np.arange(17, dtype=np.float32), (P, 17)).copy()
    return code2, iota17


def build_nc(rows=ROWS):
    nt = rows // P
    assert sum(CHUNKS) == nt
    MAXC = max(CHUNKS)

    AL = mybir.AluOpType
    nc = bacc.Bacc()
    # ab is a+b stacked host-side: rows [0, rows) = a, [rows, 2*rows) = b
    ab_d = nc.declare_dram_parameter("ab", [2 * rows, F], FP, isOutput=False)
    code_d = nc.declare_dram_parameter("code2", [P, 512], FP, isOutput=False)
    iota_d = nc.declare_dram_parameter("iota17", [P, 17], FP, isOutput=False)
    out_d = nc.declare_dram_parameter("out", [rows, F], FP, isOutput=True)

    # [p, j(a/b), t, f] view of the input tiles, partition dim first
    ab_j = ab_d[:, :].rearrange("(j t p) f -> p j t f", j=2, p=P)

    def out_view(g0, ntiles):
        return out_d[g0 * P : (g0 + ntiles) * P, :].rearrange(
            "(t p) f -> p t f", t=ntiles, p=P
        )

    with TileContext(nc) as tc:
        with (
            tc.tile_pool(name="consts", bufs=1) as cpool,
            tc.tile_pool(name="io", bufs=5) as iopool,
            tc.tile_pool(name="sums", bufs=3) as spool,
            tc.tile_pool(name="scratch", bufs=4) as scpool,
            tc.tile_pool(name="arrs", bufs=2) as apool,
            tc.tile_pool(name="outp", bufs=2) as opool,
        ):
            code_raw = cpool.tile([P, 512], FP, tag="code_raw")
            code2 = cpool.tile([P, 512], FP, tag="code2")
            iota_raw = cpool.tile([P, 17], FP, tag="iota_raw")
            iota17 = cpool.tile([P, 17], FP, tag="iota17")
            nc.sync.dma_start(code_raw[:, :], code_d[:, :])
            nc.sync.dma_start(iota_raw[:, :], iota_d[:, :])
            # pre-touch consts on DVE so compute ops only wait on DVE state
            nc.vector.tensor_copy(code2[:, :], code_raw[:, :])
            nc.vector.tensor_copy(iota17[:, :], iota_raw[:, :])

            ch_base = 0
            for ch, ntc in enumerate(CHUNKS):
                last_ch = ch == len(CHUNKS) - 1
                z2 = apool.tile([P, 2 * MAXC], FP, tag="z2")
                z2_i = apool.tile([P, 2 * MAXC], I32, tag="z2i")
                xnib_i = apool.tile([P, 8 * MAXC], I32, tag="xnibi")
                xnib = apool.tile([P, 8 * MAXC], FP, tag="xnib")
                chist = apool.tile([P, 9 * MAXC], FP, tag="chist")
                y_all = apool.tile([P, 8 * MAXC], FP, tag="yall")
                wrap = apool.tile([P, 8 * MAXC], FP, tag="wrap")
                ymod = apool.tile([P, 8 * MAXC], FP, tag="ymod")
                dtile = apool.tile([P, 8 * MAXC * 17], FP, tag="dtile")
                s17 = apool.tile([P, 8 * MAXC * 17], FP, tag="s17")

                # ---- phase 1: loads (one a+b stream), s = a+b, dots ----
                t = 0
                for bsz in BATCHES[ch]:
                    g0 = ch_base + t
                    ab_b = iopool.tile([P, 2 * BMAX * F], FP, tag="ab")
                    abv = ab_b[:, : 2 * bsz * F].rearrange(
                        "p (j t f) -> p j t f", j=2, t=bsz
                    )
                    nc.sync.dma_start(abv[:, 0], ab_j[:, 0, g0 : g0 + bsz, :])
                    nc.sync.dma_start(abv[:, 1], ab_j[:, 1, g0 : g0 + bsz, :])
                    # bf16 s: values {0,1,2} exact; a single non-fp32 STT source
                    # frees a DVE read port (full-rate dot, no shared-pair grab)
                    s_b = spool.tile([P, BMAX * F], BF, tag="sb")
                    for ti in range(bsz):
                        nc.vector.tensor_add(
                            s_b[:, ti * F : (ti + 1) * F],
                            ab_b[:, ti * F : (ti + 1) * F],
                            ab_b[:, (bsz + ti) * F : (bsz + ti + 1) * F],
                        )
                        for i2 in range(2):
                            prod = scpool.tile([P, 512], FP, tag="prod")
                            nc.vector.scalar_tensor_tensor(
                                out=prod[:, :],
                                in0=s_b[:, ti * F + i2 * 512 : ti * F + (i2 + 1) * 512],
                                scalar=1.0,
                                in1=code2[:, :],
                                op0=AL.mult,
                                op1=AL.mult,
                                accum_out=z2[:, i2 * ntc + t + ti : i2 * ntc + t + ti + 1],
                            )
                    t += bsz

                # ---- phase 2: unpack four 6-bit fields -> xnib ----
                nc.vector.tensor_copy(z2_i[:, : 2 * ntc], z2[:, : 2 * ntc])
                z2_v = z2_i[:, : 2 * ntc].rearrange("p (i2 t) -> p i2 t", t=ntc)
                xn_v = xnib_i[:, : 8 * ntc].rearrange(
                    "p (i2 k t) -> p i2 k t", i2=2, k=4
                )
                nc.vector.tensor_scalar(
                    out=xn_v[:, :, 0, :], in0=z2_v, scalar1=63, scalar2=None,
                    op0=AL.bitwise_and,
                )
                nc.vector.tensor_scalar(
                    out=xn_v[:, :, 1, :], in0=z2_v, scalar1=6, scalar2=63,
                    op0=AL.logical_shift_right, op1=AL.bitwise_and,
                )
                nc.vector.tensor_scalar(
                    out=xn_v[:, :, 2, :], in0=z2_v, scalar1=12, scalar2=63,
                    op0=AL.logical_shift_right, op1=AL.bitwise_and,
                )
                nc.vector.tensor_scalar(
                    out=xn_v[:, :, 3, :], in0=z2_v, scalar1=18, scalar2=None,
                    op0=AL.logical_shift_right,
                )
                nc.vector.tensor_copy(xnib[:, : 8 * ntc], xnib_i[:, : 8 * ntc])

                # ---- phase 3: carry chain, shifted carry c~ = c + 15 ----
                nc.vector.memset(chist[:, 0:ntc], 15.5)
                for n in range(8):
                    y_n = y_all[:, n * ntc : (n + 1) * ntc]
                    nc.vector.scalar_tensor_tensor(
                        out=y_n, in0=xnib[:, n * ntc : (n + 1) * ntc],
                        scalar=-15.0, in1=chist[:, n * ntc : (n + 1) * ntc],
                        op0=AL.add, op1=AL.add,
                    )
                    nc.vector.tensor_scalar(
                        out=chist[:, (n + 1) * ntc : (n + 2) * ntc], in0=y_n,
                        scalar1=15.0, scalar2=16.0, op0=AL.max, op1=AL.min,
                    )

                # ---- phase 4: wrap to ymod in [0, 15.5] ----
                nc.vector.tensor_scalar(
                    out=wrap[:, : 8 * ntc], in0=y_all[:, : 8 * ntc],
                    scalar1=15.75, scalar2=None, op0=AL.is_ge,
                )
                nc.vector.scalar_tensor_tensor(
                    out=ymod[:, : 8 * ntc], in0=wrap[:, : 8 * ntc], scalar=-16.0,
                    in1=y_all[:, : 8 * ntc], op0=AL.mult, op1=AL.add,
                )

                # ---- phase 5: soft dists s17 = relu(1 - |ymod - j|) ----
                # |.| and relu run on the otherwise-idle ACT engine
                G = 8 * ntc
                s_v = s17[:, : G * 17].rearrange("p (g k) -> p g k", k=17)
                d_v = dtile[:, : G * 17].rearrange("p (g k) -> p g k", k=17)
                ymod_b = ymod[:, :G, None].broadcast_to([P, G, 17])
                iota_b = iota17[:, None, :].broadcast_to([P, G, 17])
                nc.vector.tensor_tensor(d_v, ymod_b, iota_b, op=AL.subtract)
                nc.scalar.activation(
                    dtile[:, : G * 17], dtile[:, : G * 17],
                    mybir.ActivationFunctionType.Abs,
                )
                nc.scalar.activation(
                    s17[:, : G * 17], dtile[:, : G * 17],
                    mybir.ActivationFunctionType.Relu, bias=1.0, scale=-1.0,
                )
                # wraparound fold: dist[0] += s17[16]
                nc.vector.tensor_add(
                    s_v[:, :, 0:1], s_v[:, :, 0:1], s_v[:, :, 16:17]
                )

                # ---- phase 6: outer products + stores ----
                s5 = s17[:, : G * 17].rearrange(
                    "p (i two t k) -> p i two t k", two=2, t=ntc, k=17
                )
                dve_t0 = ntc - DVE_TAIL_OUTERS if last_ch else ntc
                for t0 in range(0, ntc, 4):
                    o4 = opool.tile([P, 4 * F], FP, tag="o4")
                    for ti in range(4):
                        t = t0 + ti
                        o_vv = o4[:, ti * F : (ti + 1) * F].rearrange(
                            "p (i h l) -> p i h l", h=16, l=16
                        )
                        h_b = s5[:, :, 1, t, 0:16][:, :, :, None].broadcast_to(
                            [P, 4, 16, 16])
                        l_b = s5[:, :, 0, t, 0:16][:, :, None, :].broadcast_to(
                            [P, 4, 16, 16])
                        eng = nc.vector if t >= dve_t0 else nc.gpsimd
                        eng.tensor_mul(o_vv, h_b, l_b)
                    nc.scalar.dma_start(
                        out_view(ch_base + t0, 4),
                        o4[:, :].rearrange("p (t f) -> p t f", t=4),
                    )
                ch_base += ntc

    nc.finalize()
    return nc


_NC_CACHE = {}
LAST_RESULT = None


def kernel(**inputs) -> np.ndarray:
    global LAST_RESULT
    a = np.asarray(inputs["a"], dtype=np.float32).reshape(B_FULL, F)
    b = np.asarray(inputs["b"], dtype=np.float32).reshape(B_FULL, F)
    code2, iota17 = _const_tables()

    if ROWS not in _NC_CACHE:
        _NC_CACHE[ROWS] = build_nc(ROWS)
    nc = _NC_CACHE[ROWS]

    in_maps = []
    for c in range(N_CORES):
        ab = np.concatenate(
            [a[c * ROWS : (c + 1) * ROWS], b[c * ROWS : (c + 1) * ROWS]], axis=0
        )
        in_maps.append({
            "ab": np.ascontiguousarray(ab),
            "code2": code2,
            "iota17": iota17,
        })
    res = run_bass_kernel_spmd(nc, in_maps, core_ids=list(range(N_CORES)))
    LAST_RESULT = res
    out = np.concatenate([r["out"] for r in res.results], axis=0)
    return out.reshape(B_FULL, 4, 256)


# revision 13
# speedup vs baseline: 1.1304x; 1.0993x over previous
"""Trainium2 Bass kernel for nn_NeuralALU (batched byte-encoded 32-bit add).

The reference network computes, per batch element, a chain of table-lookup
matmuls + sharp softmaxes (scale=100) over exactly-one-hot byte encodings.
Because the inputs are exact one-hots, the float pipeline collapses to a
discrete algorithm (validated to ~1e-22 rel-err):

  s = a + b                      per 256-wide byte block (<=2 nonzeros)
  z = dot(s, code2)              code2 packs lo/hi nibble sums of 2 bytes
                                 into four 6-bit fields (exact in f32)
  xnib[n] in [0,30]              per-nibble sums, carry order lo0,hi0,...
  soft carry chain:  Y_n = x_n + c_n,  c_{n+1} = clamp(Y_n - 15, 0, 1)
                     (kept shifted: c~ = c + 15 so both steps are 1 op)
  soft dist:  ymod = Y - 16*[Y >= 15.75]
              s17[j] = relu(1 - |ymod - j|),  j = 0..16   (17-wide)
              dist[k] = s17[k],  dist[0] += s17[16]       (wraparound)
       This one triangle kernel reproduces onehot(U)*(1-P/2) +
       onehot((U+1)%16)*(P/2) for carry states c in {0, 0.5, 1}.
  out byte row [256] = outer(h_dist, l_dist) flattened

Engine plan (per core, pure data-parallel over batch):
  - DVE: s=a+b (in-place), packed dots, field extraction, carry chain,
    dist prep, and a contiguous tail block of the last chunk's outer
    products (tail shortening).
  - Pool (gpsimd): all other outer products.
  - ACT (scalar): |.| and relu of the soft dists; issues output stores
    on its own HWDGE ring so loads and stores never share a FIFO.
  - Inputs batched 2MB per stream; first batches smaller to cut pipeline
    lead-in.
"""

import numpy as np

import concourse.bass as bass
import concourse.bacc as bacc
import concourse.mybir as mybir
from concourse.tile import TileContext
from concourse.bass_utils import run_bass_kernel_spmd

N_CORES = 8
B_FULL = 32768
ROWS = B_FULL // N_CORES  # 4096 rows per core
F = 1024  # 4 bytes x 256 one-hot
P = 128
CHUNKS = (16, 16)
BATCHES = ((2, 2, 4, 4, 4), (4, 4, 4, 4))  # input batch plan per chunk
BMAX = 4
DVE_TAIL_OUTERS = 6  # last chunk: final N tiles' outers run on DVE

FP = mybir.dt.float32
I32 = mybir.dt.int32


def _const_tables():
    k = np.arange(256)
    nib = ((k % 16) + 64.0 * (k // 16)).astype(np.float32)  # <= 975
    # two bytes per dot: second byte's fields scaled by 4096 (sums stay
    # exact in f32: max 2*975*4096 + 2*975 < 2^23)
    code2 = np.concatenate([nib, nib * 4096.0])  # [512]
    code2 = np.broadcast_to(code2, (P, 512)).copy()
    iota17 = np.broadcast_to(np.arange(17, dtype=np.float32), (P, 17)).copy()
    return code2, iota17


def build_nc(rows=ROWS):
    nt = rows // P
    assert sum(CHUNKS) == nt
    MAXC = max(CHUNKS)

    AL = mybir.AluOpType
    nc = bacc.Bacc()
    a_d = nc.declare_dram_parameter("a", [rows, F], FP, isOutput=False)
    b_d = nc.declare_dram_parameter("b", [rows, F], FP, isOutput=False)
    code_d = nc.declare_dram_parameter("code2", [P, 512], FP, isOutput=False)
    iota_d = nc.declare_dram_parameter("iota17", [P, 17], FP, isOutput=False)
    out_d = nc.declare_dram_parameter("out", [rows, F], FP, isOutput=True)

    def dram_view(d, g0, ntiles):  # [p, t, f] view of tiles [g0, g0+ntiles)
        return d[g0 * P : (g0 + ntiles) * P, :].rearrange(
            "(t p) f -> p t f", t=ntiles, p=P
        )

    with TileContext(nc) as tc:
        with (
            tc.tile_pool(name="consts", bufs=1) as cpool,
            tc.tile_pool(name="io", bufs=3) as iopool,
            tc.tile_pool(name="scratch", bufs=4) as scpool,
            tc.tile_pool(name="arrs", bufs=2) as apool,
            tc.tile_pool(name="outp", bufs=2) as opool,
        ):
            code_raw = cpool.tile([P, 512], FP, tag="code_raw")
            code2 = cpool.tile([P, 512], FP, tag="code2")
            iota_raw = cpool.tile([P, 17], FP, tag="iota_raw")
            iota17 = cpool.tile([P, 17], FP, tag="iota17")
            nc.sync.dma_start(code_raw[:, :], code_d[:, :])
            nc.sync.dma_start(iota_raw[:, :], iota_d[:, :])
            # pre-touch consts on DVE so compute ops only wait on DVE state
            nc.vector.tensor_copy(code2[:, :], code_raw[:, :])
            nc.vector.tensor_copy(iota17[:, :], iota_raw[:, :])

            ch_base = 0
            for ch, ntc in enumerate(CHUNKS):
                last_ch = ch == len(CHUNKS) - 1
                z2 = apool.tile([P, 2 * MAXC], FP, tag="z2")
                z2_i = apool.tile([P, 2 * MAXC], I32, tag="z2i")
                xnib_i = apool.tile([P, 8 * MAXC], I32, tag="xnibi")
                xnib = apool.tile([P, 8 * MAXC], FP, tag="xnib")
                chist = apool.tile([P, 9 * MAXC], FP, tag="chist")
                y_all = apool.tile([P, 8 * MAXC], FP, tag="yall")
                wrap = apool.tile([P, 8 * MAXC], FP, tag="wrap")
                ymod = apool.tile([P, 8 * MAXC], FP, tag="ymod")
                dtile = apool.tile([P, 8 * MAXC * 17], FP, tag="dtile")
                s17 = apool.tile([P, 8 * MAXC * 17], FP, tag="s17")

                # ---- phase 1: batched loads, s = a+b (in-place), dots ----
                t = 0
                for bsz in BATCHES[ch]:
                    g0 = ch_base + t
                    a_b = iopool.tile([P, BMAX * F], FP, tag="ab")
                    b_b = iopool.tile([P, BMAX * F], FP, tag="bb")
                    nc.sync.dma_start(
                        a_b[:, : bsz * F].rearrange("p (t f) -> p t f", t=bsz),
                        dram_view(a_d, g0, bsz),
                    )
                    nc.sync.dma_start(
                        b_b[:, : bsz * F].rearrange("p (t f) -> p t f", t=bsz),
                        dram_view(b_d, g0, bsz),
                    )
                    for ti in range(bsz):
                        sl = slice(ti * F, (ti + 1) * F)
                        nc.vector.tensor_add(a_b[:, sl], a_b[:, sl], b_b[:, sl])
                        for i2 in range(2):
                            prod = scpool.tile([P, 512], FP, tag="prod")
                            nc.vector.scalar_tensor_tensor(
                                out=prod[:, :],
                                in0=a_b[:, ti * F + i2 * 512 : ti * F + (i2 + 1) * 512],
                                scalar=1.0,
                                in1=code2[:, :],
                                op0=AL.mult,
                                op1=AL.mult,
                                accum_out=z2[:, i2 * ntc + t + ti : i2 * ntc + t + ti + 1],
                            )
                    t += bsz

                # ---- phase 2: unpack four 6-bit fields -> xnib ----
                nc.vector.tensor_copy(z2_i[:, : 2 * ntc], z2[:, : 2 * ntc])
                z2_v = z2_i[:, : 2 * ntc].rearrange("p (i2 t) -> p i2 t", t=ntc)
                xn_v = xnib_i[:, : 8 * ntc].rearrange(
                    "p (i2 k t) -> p i2 k t", i2=2, k=4
                )
                nc.vector.tensor_scalar(
                    out=xn_v[:, :, 0, :], in0=z2_v, scalar1=63, scalar2=None,
                    op0=AL.bitwise_and,
                )
                nc.vector.tensor_scalar(
                    out=xn_v[:, :, 1, :], in0=z2_v, scalar1=6, scalar2=63,
                    op0=AL.logical_shift_right, op1=AL.bitwise_and,
                )
                nc.vector.tensor_scalar(
                    out=xn_v[:, :, 2, :], in0=z2_v, scalar1=12, scalar2=63,
                    op0=AL.logical_shift_right, op1=AL.bitwise_and,
                )
                nc.vector.tensor_scalar(
                    out=xn_v[:, :, 3, :], in0=z2_v, scalar1=18, scalar2=None,
                    op0=AL.logical_shift_right,
                )
                nc.vector.tensor_copy(xnib[:, : 8 * ntc], xnib_i[:, : 8 * ntc])

                # ---- phase 3: carry chain, shifted carry c~ = c + 15 ----
                nc.vector.memset(chist[:, 0:ntc], 15.5)
                for n in range(8):
                    y_n = y_all[:, n * ntc : (n + 1) * ntc]
                    nc.vector.scalar_tensor_tensor(
                        out=y_n, in0=xnib[:, n * ntc : (n + 1) * ntc],
                        scalar=-15.0, in1=chist[:, n * ntc : (n + 1) * ntc],
                        op0=AL.add, op1=AL.add,
                    )
                    nc.vector.tensor_scalar(
                        out=chist[:, (n + 1) * ntc : (n + 2) * ntc], in0=y_n,
                        scalar1=15.0, scalar2=16.0, op0=AL.max, op1=AL.min,
                    )

                # ---- phase 4: wrap to ymod in [0, 15.5] ----
                nc.vector.tensor_scalar(
                    out=wrap[:, : 8 * ntc], in0=y_all[:, : 8 * ntc],
                    scalar1=15.75, scalar2=None, op0=AL.is_ge,
                )
                nc.vector.scalar_tensor_tensor(
                    out=ymod[:, : 8 * ntc], in0=wrap[:, : 8 * ntc], scalar=-16.0,
                    in1=y_all[:, : 8 * ntc], op0=AL.mult, op1=AL.add,
                )

                # ---- phase 5: soft dists s17 = relu(1 - |ymod - j|) ----
                # |.| and relu run on the otherwise-idle ACT engine
                G = 8 * ntc
                s_v = s17[:, : G * 17].rearrange("p (g k) -> p g k", k=17)
                d_v = dtile[:, : G * 17].rearrange("p (g k) -> p g k", k=17)
                ymod_b = ymod[:, :G, None].broadcast_to([P, G, 17])
                iota_b = iota17[:, None, :].broadcast_to([P, G, 17])
                nc.vector.tensor_tensor(d_v, ymod_b, iota_b, op=AL.subtract)
                nc.scalar.activation(
                    dtile[:, : G * 17], dtile[:, : G * 17],
                    mybir.ActivationFunctionType.Abs,
                )
                nc.scalar.activation(
                    s17[:, : G * 17], dtile[:, : G * 17],
                    mybir.ActivationFunctionType.Relu, bias=1.0, scale=-1.0,
                )
                # wraparound fold: dist[0] += s17[16]
                nc.vector.tensor_add(
                    s_v[:, :, 0:1], s_v[:, :, 0:1], s_v[:, :, 16:17]
                )

                # ---- phase 6: outer products + stores ----
                s5 = s17[:, : G * 17].rearrange(
                    "p (i two t k) -> p i two t k", two=2, t=ntc, k=17
                )
                dve_t0 = ntc - DVE_TAIL_OUTERS if last_ch else ntc
                for t0 in range(0, ntc, 4):
                    o4 = opool.tile([P, 4 * F], FP, tag="o4")
                    for ti in range(4):
                        t = t0 + ti
                        o_vv = o4[:, ti * F : (ti + 1) * F].rearrange(
                            "p (i h l) -> p i h l", h=16, l=16
                        )
                        h_b = s5[:, :, 1, t, 0:16][:, :, :, None].broadcast_to(
                            [P, 4, 16, 16])
                        l_b = s5[:, :, 0, t, 0:16][:, :, None, :].broadcast_to(
                            [P, 4, 16, 16])
                        eng = nc.vector if t >= dve_t0 else nc.gpsimd
                        eng.tensor_mul(o_vv, h_b, l_b)
                    nc.scalar.dma_start(
                        dram_view(out_d, ch_base + t0, 4),
                        o4[:, :].rearrange("p (t f) -> p t f", t=4),
                    )
                ch_base += ntc

    nc.finalize()
    return nc


_NC_CACHE = {}
LAST_RESULT = None


def kernel(**inputs) -> np.ndarray:
    global LAST_RESULT
    a = np.ascontiguousarray(np.asarray(inputs["a"], dtype=np.float32)).reshape(B_FULL, F)
    b = np.ascontiguousarray(np.asarray(inputs["b"], dtype=np.float32)).reshape(B_FULL, F)
    code2, iota17 = _const_tables()

    if ROWS not in _NC_CACHE:
        _NC_CACHE[ROWS] = build_nc(ROWS)
    nc = _NC_CACHE[ROWS]

    in_maps = []
    for c in range(N_CORES):
        in_maps.append({
            "a": a[c * ROWS : (c + 1) * ROWS],
            "b": b[c * ROWS : (c + 1) * ROWS],
            "code2": code2,
            "iota17": iota17,
        })
    res = run_bass_kernel_spmd(nc, in_maps, core_ids=list(range(N_CORES)))
    LAST_RESULT = res
    out = np.concatenate([r["out"] for r in res.results], axis=0)
    return out.reshape(B_FULL, 4, 256)


# revision 14
# speedup vs baseline: 1.2800x; 1.1323x over previous
"""Trainium2 Bass kernel for nn_NeuralALU (batched byte-encoded 32-bit add).

The reference network computes, per batch element, a chain of table-lookup
matmuls + sharp softmaxes (scale=100) over exactly-one-hot byte encodings.
Because the inputs are exact one-hots, the float pipeline collapses to a
discrete algorithm (validated to ~1e-22 rel-err):

  z  = dot(a, code2) + dot(b, code2)   per byte-pair; code2 packs the
       lo/hi nibble codes of 2 bytes into four 6-bit fields (f32-exact)
  xnib[n] in [0,30]              per-nibble sums, carry order lo0,hi0,...
  soft carry chain:  Y_n = x_n + c_n,  c_{n+1} = clamp(Y_n - 15, 0, 1)
                     (kept shifted: c~ = c + 15 so both steps are 1 op)
  soft dist:  ymod = Y - 16*[Y >= 15.75]
              s17[j] = relu(1 - |ymod - j|),  j = 0..16   (17-wide)
              dist[k] = s17[k],  dist[0] += s17[16]       (wraparound)
       This one triangle kernel reproduces onehot(U)*(1-P/2) +
       onehot((U+1)%16)*(P/2) for carry states c in {0, 0.5, 1}.
  out byte row [256] = outer(h_dist, l_dist) flattened

Engine/memory plan (per core, pure data-parallel over batch):
  - DVE's 2nd read port is the SBUF port pair it shares with GpSimd
    (exclusive lock, full-instruction hold), so every dual-SBUF-source
    DVE op running concurrently with a Pool op stalls one of the two.
    To keep the phase-1 stream collision-free, the constant tables live
    in PSUM (DVE reads them via its separate PSUM port), and s=a+b is
    folded into the dots (dot is linear), so phase 1 has no
    dual-SBUF-source ops at all.
  - Pool (gpsimd) computes chunk 0's outer products concurrently with
    chunk 1's phase 1 (no shared-pair conflict); DVE computes chunk 1's
    outer products in the tail while Pool is already done.
  - ACT (scalar): |.| and relu of the soft dists; issues output stores
    on its own HWDGE ring so loads and stores never share a FIFO.
"""

import numpy as np

import concourse.bass as bass
import concourse.bacc as bacc
import concourse.mybir as mybir
from concourse.tile import TileContext
from concourse.bass_utils import run_bass_kernel_spmd

N_CORES = 8
B_FULL = 32768
ROWS = B_FULL // N_CORES  # 4096 rows per core
F = 1024  # 4 bytes x 256 one-hot
P = 128
CHUNKS = (16, 16)
BATCHES = ((2, 2, 4, 4, 4), (4, 4, 4, 4))  # input batch plan per chunk
BMAX = 4

FP = mybir.dt.float32
I32 = mybir.dt.int32


def _const_tables():
    k = np.arange(256)
    nib = ((k % 16) + 64.0 * (k // 16)).astype(np.float32)  # <= 975
    # two bytes per dot: second byte's fields scaled by 4096 (sums stay
    # exact in f32: max 2*975*4096 + 2*975 < 2^23)
    code2 = np.concatenate([nib, nib * 4096.0])  # [512]
    code2 = np.broadcast_to(code2, (P, 512)).copy()
    iota17 = np.broadcast_to(np.arange(17, dtype=np.float32), (P, 17)).copy()
    return code2, iota17


def build_nc(rows=ROWS):
    nt = rows // P
    assert sum(CHUNKS) == nt
    MAXC = max(CHUNKS)

    AL = mybir.AluOpType
    nc = bacc.Bacc()
    a_d = nc.declare_dram_parameter("a", [rows, F], FP, isOutput=False)
    b_d = nc.declare_dram_parameter("b", [rows, F], FP, isOutput=False)
    code_d = nc.declare_dram_parameter("code2", [P, 512], FP, isOutput=False)
    iota_d = nc.declare_dram_parameter("iota17", [P, 17], FP, isOutput=False)
    out_d = nc.declare_dram_parameter("out", [rows, F], FP, isOutput=True)

    def dram_view(d, g0, ntiles):  # [p, t, f] view of tiles [g0, g0+ntiles)
        return d[g0 * P : (g0 + ntiles) * P, :].rearrange(
            "(t p) f -> p t f", t=ntiles, p=P
        )

    with TileContext(nc) as tc:
        with (
            tc.tile_pool(name="consts", bufs=1) as cpool,
            tc.tile_pool(name="pconsts", bufs=1, space="PSUM") as ppool,
            tc.tile_pool(name="io", bufs=3) as iopool,
            tc.tile_pool(name="scratch", bufs=4) as scpool,
            tc.tile_pool(name="arrs", bufs=2) as apool,
            tc.tile_pool(name="outp", bufs=3) as opool,
        ):
            code_raw = cpool.tile([P, 512], FP, tag="code_raw")
            iota_raw = cpool.tile([P, 17], FP, tag="iota_raw")
            # tables live in PSUM: DVE reads them on its dedicated PSUM
            # port, so phase-1 ops need no second SBUF port
            code2 = ppool.tile([P, 512], FP, tag="code2")
            iota17 = ppool.tile([P, 17], FP, tag="iota17")
            nc.sync.dma_start(code_raw[:, :], code_d[:, :])
            nc.sync.dma_start(iota_raw[:, :], iota_d[:, :])
            nc.vector.tensor_copy(code2[:, :], code_raw[:, :])
            nc.vector.tensor_copy(iota17[:, :], iota_raw[:, :])

            ch_base = 0
            for ch, ntc in enumerate(CHUNKS):
                last_ch = ch == len(CHUNKS) - 1
                z2a = apool.tile([P, 2 * MAXC], FP, tag="z2a")
                z2b = apool.tile([P, 2 * MAXC], FP, tag="z2b")
                z2 = apool.tile([P, 2 * MAXC], FP, tag="z2")
                z2_i = apool.tile([P, 2 * MAXC], I32, tag="z2i")
                xnib_i = apool.tile([P, 8 * MAXC], I32, tag="xnibi")
                xnib = apool.tile([P, 8 * MAXC], FP, tag="xnib")
                chist = apool.tile([P, 9 * MAXC], FP, tag="chist")
                y_all = apool.tile([P, 8 * MAXC], FP, tag="yall")
                wrap = apool.tile([P, 8 * MAXC], FP, tag="wrap")
                ymod = apool.tile([P, 8 * MAXC], FP, tag="ymod")
                dtile = apool.tile([P, 8 * MAXC * 17], FP, tag="dtile")
                s17 = apool.tile([P, 8 * MAXC * 17], FP, tag="s17")

                # ---- phase 1: batched loads + 4 single-source dots/tile ----
                t = 0
                for bsz in BATCHES[ch]:
                    g0 = ch_base + t
                    a_b = iopool.tile([P, BMAX * F], FP, tag="ab")
                    b_b = iopool.tile([P, BMAX * F], FP, tag="bb")
                    nc.sync.dma_start(
                        a_b[:, : bsz * F].rearrange("p (t f) -> p t f", t=bsz),
                        dram_view(a_d, g0, bsz),
                    )
                    nc.sync.dma_start(
                        b_b[:, : bsz * F].rearrange("p (t f) -> p t f", t=bsz),
                        dram_view(b_d, g0, bsz),
                    )
                    for ti in range(bsz):
                        for src, zdst in ((a_b, z2a), (b_b, z2b)):
                            for i2 in range(2):
                                prod = scpool.tile([P, 512], FP, tag="prod")
                                nc.vector.scalar_tensor_tensor(
                                    out=prod[:, :],
                                    in0=src[:, ti * F + i2 * 512 : ti * F + (i2 + 1) * 512],
                                    scalar=1.0,
                                    in1=code2[:, :],
                                    op0=AL.mult,
                                    op1=AL.mult,
                                    accum_out=zdst[:, i2 * ntc + t + ti : i2 * ntc + t + ti + 1],
                                )
                    t += bsz

                # ---- phase 2: z = za + zb, unpack four 6-bit fields ----
                nc.vector.tensor_add(
                    z2[:, : 2 * ntc], z2a[:, : 2 * ntc], z2b[:, : 2 * ntc]
                )
                nc.vector.tensor_copy(z2_i[:, : 2 * ntc], z2[:, : 2 * ntc])
                z2_v = z2_i[:, : 2 * ntc].rearrange("p (i2 t) -> p i2 t", t=ntc)
                xn_v = xnib_i[:, : 8 * ntc].rearrange(
                    "p (i2 k t) -> p i2 k t", i2=2, k=4
                )
                nc.vector.tensor_scalar(
                    out=xn_v[:, :, 0, :], in0=z2_v, scalar1=63, scalar2=None,
                    op0=AL.bitwise_and,
                )
                nc.vector.tensor_scalar(
                    out=xn_v[:, :, 1, :], in0=z2_v, scalar1=6, scalar2=63,
                    op0=AL.logical_shift_right, op1=AL.bitwise_and,
                )
                nc.vector.tensor_scalar(
                    out=xn_v[:, :, 2, :], in0=z2_v, scalar1=12, scalar2=63,
                    op0=AL.logical_shift_right, op1=AL.bitwise_and,
                )
                nc.vector.tensor_scalar(
                    out=xn_v[:, :, 3, :], in0=z2_v, scalar1=18, scalar2=None,
                    op0=AL.logical_shift_right,
                )
                nc.vector.tensor_copy(xnib[:, : 8 * ntc], xnib_i[:, : 8 * ntc])

                # ---- phase 3: carry chain, shifted carry c~ = c + 15 ----
                nc.vector.memset(chist[:, 0:ntc], 15.5)
                for n in range(8):
                    y_n = y_all[:, n * ntc : (n + 1) * ntc]
                    nc.vector.scalar_tensor_tensor(
                        out=y_n, in0=xnib[:, n * ntc : (n + 1) * ntc],
                        scalar=-15.0, in1=chist[:, n * ntc : (n + 1) * ntc],
                        op0=AL.add, op1=AL.add,
                    )
                    nc.vector.tensor_scalar(
                        out=chist[:, (n + 1) * ntc : (n + 2) * ntc], in0=y_n,
                        scalar1=15.0, scalar2=16.0, op0=AL.max, op1=AL.min,
                    )

                # ---- phase 4: wrap to ymod in [0, 15.5] ----
                nc.vector.tensor_scalar(
                    out=wrap[:, : 8 * ntc], in0=y_all[:, : 8 * ntc],
                    scalar1=15.75, scalar2=None, op0=AL.is_ge,
                )
                nc.vector.scalar_tensor_tensor(
                    out=ymod[:, : 8 * ntc], in0=wrap[:, : 8 * ntc], scalar=-16.0,
                    in1=y_all[:, : 8 * ntc], op0=AL.mult, op1=AL.add,
                )

                # ---- phase 5: soft dists s17 = relu(1 - |ymod - j|) ----
                # iota from PSUM (single SBUF source); |.|/relu on ACT
                G = 8 * ntc
                s_v = s17[:, : G * 17].rearrange("p (g k) -> p g k", k=17)
                d_v = dtile[:, : G * 17].rearrange("p (g k) -> p g k", k=17)
                ymod_b = ymod[:, :G, None].broadcast_to([P, G, 17])
                iota_b = iota17[:, None, :].broadcast_to([P, G, 17])
                nc.vector.tensor_tensor(d_v, ymod_b, iota_b, op=AL.subtract)
                nc.scalar.activation(
                    dtile[:, : G * 17], dtile[:, : G * 17],
                    mybir.ActivationFunctionType.Abs,
                )
                nc.scalar.activation(
                    s17[:, : G * 17], dtile[:, : G * 17],
                    mybir.ActivationFunctionType.Relu, bias=1.0, scale=-1.0,
                )
                # wraparound fold: dist[0] += s17[16]
                nc.vector.tensor_add(
                    s_v[:, :, 0:1], s_v[:, :, 0:1], s_v[:, :, 16:17]
                )

                # ---- phase 6: outer products + stores ----
                # chunk 0 outers on Pool (overlap chunk 1 phase 1, which is
                # single-source and so never contends for the shared pair);
                # last chunk outers on DVE alone (Pool already finished)
                s5 = s17[:, : G * 17].rearrange(
                    "p (i two t k) -> p i two t k", two=2, t=ntc, k=17
                )
                eng = nc.vector if last_ch else nc.gpsimd
                for t0 in range(0, ntc, 4):
                    o4 = opool.tile([P, 4 * F], FP, tag="o4")
                    for ti in range(4):
                        t = t0 + ti
                        o_vv = o4[:, ti * F : (ti + 1) * F].rearrange(
                            "p (i h l) -> p i h l", h=16, l=16
                        )
                        h_b = s5[:, :, 1, t, 0:16][:, :, :, None].broadcast_to(
                            [P, 4, 16, 16])
                        l_b = s5[:, :, 0, t, 0:16][:, :, None, :].broadcast_to(
                            [P, 4, 16, 16])
                        eng.tensor_mul(o_vv, h_b, l_b)
                    nc.scalar.dma_start(
                        dram_view(out_d, ch_base + t0, 4),
                        o4[:, :].rearrange("p (t f) -> p t f", t=4),
                    )
                ch_base += ntc

    nc.finalize()
    return nc


_NC_CACHE = {}
LAST_RESULT = None


def kernel(**inputs) -> np.ndarray:
    global LAST_RESULT
    a = np.ascontiguousarray(np.asarray(inputs["a"], dtype=np.float32)).reshape(B_FULL, F)
    b = np.ascontiguousarray(np.asarray(inputs["b"], dtype=np.float32)).reshape(B_FULL, F)
    code2, iota17 = _const_tables()

    if ROWS not in _NC_CACHE:
        _NC_CACHE[ROWS] = build_nc(ROWS)
    nc = _NC_CACHE[ROWS]

    in_maps = []
    for c in range(N_CORES):
        in_maps.append({
            "a": a[c * ROWS : (c + 1) * ROWS],
            "b": b[c * ROWS : (c + 1) * ROWS],
            "code2": code2,
            "iota17": iota17,
        })
    res = run_bass_kernel_spmd(nc, in_maps, core_ids=list(range(N_CORES)))
    LAST_RESULT = res
    out = np.concatenate([r["out"] for r in res.results], axis=0)
    return out.reshape(B_FULL, 4, 256)


# revision 15
# speedup vs baseline: 1.3448x; 1.0506x over previous
"""Trainium2 Bass kernel for nn_NeuralALU (batched byte-encoded 32-bit add).

The reference network computes, per batch element, a chain of table-lookup
matmuls + sharp softmaxes (scale=100) over exactly-one-hot byte encodings.
Because the inputs are exact one-hots, the float pipeline collapses to a
discrete algorithm (validated to ~1e-22 rel-err):

  z  = dot(a, code2) + dot(b, code2)   per byte-pair; code2 packs the
       lo/hi nibble codes of 2 bytes into four 6-bit fields (f32-exact)
  xnib[n] in [0,30]              per-nibble sums, carry order lo0,hi0,...
  soft carry chain:  Y_n = x_n + c_n,  c_{n+1} = clamp(Y_n - 15, 0, 1)
                     (kept shifted: c~ = c + 15 so both steps are 1 op)
  soft dist:  ymod = Y - 16*[Y >= 15.75]
              s17[j] = relu(1 - |ymod - j|),  j = 0..16   (17-wide)
              dist[k] = s17[k],  dist[0] += s17[16]       (wraparound)
       This one triangle kernel reproduces onehot(U)*(1-P/2) +
       onehot((U+1)%16)*(P/2) for carry states c in {0, 0.5, 1}.
  out byte row [256] = outer(h_dist, l_dist) flattened

Engine/memory plan (per core, pure data-parallel over batch):
  - DVE's 2nd read port is the SBUF port pair it shares with GpSimd
    (exclusive lock, full-instruction hold), so every dual-SBUF-source
    DVE op running concurrently with a Pool op stalls one of the two.
    To keep the phase-1 stream collision-free, the constant tables live
    in PSUM (DVE reads them via its separate PSUM port), and s=a+b is
    folded into the dots (dot is linear), so phase 1 has no
    dual-SBUF-source ops at all.
  - Pool (gpsimd) computes chunk 0's outer products concurrently with
    chunk 1's phase 1 (no shared-pair conflict); DVE computes chunk 1's
    outer products in the tail while Pool is already done.
  - ACT (scalar): |.| and relu of the soft dists; issues output stores
    on its own HWDGE ring so loads and stores never share a FIFO.
"""

import numpy as np

import concourse.bass as bass
import concourse.bacc as bacc
import concourse.mybir as mybir
from concourse.tile import TileContext
from concourse.bass_utils import run_bass_kernel_spmd

N_CORES = 8
B_FULL = 32768
ROWS = B_FULL // N_CORES  # 4096 rows per core
F = 1024  # 4 bytes x 256 one-hot
P = 128
CHUNKS = (16, 16)
BATCHES = ((2, 2, 4, 4, 4), (4, 4, 4, 4))  # input batch plan per chunk
BMAX = 4

FP = mybir.dt.float32
I32 = mybir.dt.int32


def _const_tables():
    k = np.arange(256)
    nib = ((k % 16) + 64.0 * (k // 16)).astype(np.float32)  # <= 975
    # two bytes per dot: second byte's fields scaled by 4096 (sums stay
    # exact in f32: max 2*975*4096 + 2*975 < 2^23)
    code2 = np.concatenate([nib, nib * 4096.0])  # [512]
    code2 = np.broadcast_to(code2, (P, 512)).copy()
    iota17 = np.broadcast_to(np.arange(17, dtype=np.float32), (P, 17)).copy()
    return code2, iota17


def build_nc(rows=ROWS):
    nt = rows // P
    assert sum(CHUNKS) == nt
    MAXC = max(CHUNKS)

    AL = mybir.AluOpType
    nc = bacc.Bacc()
    a_d = nc.declare_dram_parameter("a", [rows, F], FP, isOutput=False)
    b_d = nc.declare_dram_parameter("b", [rows, F], FP, isOutput=False)
    code_d = nc.declare_dram_parameter("code2", [P, 512], FP, isOutput=False)
    iota_d = nc.declare_dram_parameter("iota17", [P, 17], FP, isOutput=False)
    out_d = nc.declare_dram_parameter("out", [rows, F], FP, isOutput=True)

    def dram_view(d, g0, ntiles):  # [p, t, f] view of tiles [g0, g0+ntiles)
        return d[g0 * P : (g0 + ntiles) * P, :].rearrange(
            "(t p) f -> p t f", t=ntiles, p=P
        )

    with TileContext(nc) as tc:
        with (
            tc.tile_pool(name="consts", bufs=1) as cpool,
            tc.tile_pool(name="pconsts", bufs=1, space="PSUM") as ppool,
            tc.tile_pool(name="io", bufs=3) as iopool,
            tc.tile_pool(name="scratch", bufs=4) as scpool,
            tc.tile_pool(name="arrs", bufs=2) as apool,
            tc.tile_pool(name="outp", bufs=3) as opool,
        ):
            code_raw = cpool.tile([P, 512], FP, tag="code_raw")
            iota_raw = cpool.tile([P, 17], FP, tag="iota_raw")
            # tables live in PSUM: DVE reads them on its dedicated PSUM
            # port, so phase-1 ops need no second SBUF port
            code2 = ppool.tile([P, 512], FP, tag="code2")
            iota17 = ppool.tile([P, 17], FP, tag="iota17")
            nc.sync.dma_start(code_raw[:, :], code_d[:, :])
            nc.sync.dma_start(iota_raw[:, :], iota_d[:, :])
            nc.vector.tensor_copy(code2[:, :], code_raw[:, :])
            nc.vector.tensor_copy(iota17[:, :], iota_raw[:, :])

            ch_base = 0
            for ch, ntc in enumerate(CHUNKS):
                last_ch = ch == len(CHUNKS) - 1
                z2a = apool.tile([P, 2 * MAXC], FP, tag="z2a")
                z2b = apool.tile([P, 2 * MAXC], FP, tag="z2b")
                z2 = apool.tile([P, 2 * MAXC], FP, tag="z2")
                z2_i = apool.tile([P, 2 * MAXC], I32, tag="z2i")
                xnib_i = apool.tile([P, 8 * MAXC], I32, tag="xnibi")
                xnib = apool.tile([P, 8 * MAXC], FP, tag="xnib")
                chist = apool.tile([P, 9 * MAXC], FP, tag="chist")
                y_all = apool.tile([P, 8 * MAXC], FP, tag="yall")
                wrap = apool.tile([P, 8 * MAXC], FP, tag="wrap")
                ymod = apool.tile([P, 8 * MAXC], FP, tag="ymod")
                dtile = apool.tile([P, 8 * MAXC * 17], FP, tag="dtile")
                s17 = apool.tile([P, 8 * MAXC * 17], FP, tag="s17")

                # ---- phase 1: batched loads + 4 single-source dots/tile ----
                t = 0
                for bsz in BATCHES[ch]:
                    g0 = ch_base + t
                    a_b = iopool.tile([P, BMAX * F], FP, tag="ab")
                    b_b = iopool.tile([P, BMAX * F], FP, tag="bb")
                    nc.sync.dma_start(
                        a_b[:, : bsz * F].rearrange("p (t f) -> p t f", t=bsz),
                        dram_view(a_d, g0, bsz),
                    )
                    nc.sync.dma_start(
                        b_b[:, : bsz * F].rearrange("p (t f) -> p t f", t=bsz),
                        dram_view(b_d, g0, bsz),
                    )
                    for ti in range(bsz):
                        for src, zdst in ((a_b, z2a), (b_b, z2b)):
                            for i2 in range(2):
                                prod = scpool.tile([P, 512], FP, tag="prod")
                                nc.vector.scalar_tensor_tensor(
                                    out=prod[:, :],
                                    in0=src[:, ti * F + i2 * 512 : ti * F + (i2 + 1) * 512],
                                    scalar=1.0,
                                    in1=code2[:, :],
                                    op0=AL.mult,
                                    op1=AL.mult,
                                    accum_out=zdst[:, i2 * ntc + t + ti : i2 * ntc + t + ti + 1],
                                )
                    t += bsz

                # ---- phase 2: z = za + zb, unpack four 6-bit fields ----
                nc.vector.tensor_add(
                    z2[:, : 2 * ntc], z2a[:, : 2 * ntc], z2b[:, : 2 * ntc]
                )
                nc.vector.tensor_copy(z2_i[:, : 2 * ntc], z2[:, : 2 * ntc])
                z2_v = z2_i[:, : 2 * ntc].rearrange("p (i2 t) -> p i2 t", t=ntc)
                xn_v = xnib_i[:, : 8 * ntc].rearrange(
                    "p (i2 k t) -> p i2 k t", i2=2, k=4
                )
                nc.vector.tensor_scalar(
                    out=xn_v[:, :, 0, :], in0=z2_v, scalar1=63, scalar2=None,
                    op0=AL.bitwise_and,
                )
                nc.vector.tensor_scalar(
                    out=xn_v[:, :, 1, :], in0=z2_v, scalar1=6, scalar2=63,
                    op0=AL.logical_shift_right, op1=AL.bitwise_and,
                )
                nc.vector.tensor_scalar(
                    out=xn_v[:, :, 2, :], in0=z2_v, scalar1=12, scalar2=63,
                    op0=AL.logical_shift_right, op1=AL.bitwise_and,
                )
                nc.vector.tensor_scalar(
                    out=xn_v[:, :, 3, :], in0=z2_v, scalar1=18, scalar2=None,
                    op0=AL.logical_shift_right,
                )
                nc.vector.tensor_copy(xnib[:, : 8 * ntc], xnib_i[:, : 8 * ntc])

                # ---- phase 3: carry chain, shifted carry c~ = c + 15 ----
                nc.vector.memset(chist[:, 0:ntc], 15.5)
                for n in range(8):
                    y_n = y_all[:, n * ntc : (n + 1) * ntc]
                    nc.vector.scalar_tensor_tensor(
                        out=y_n, in0=xnib[:, n * ntc : (n + 1) * ntc],
                        scalar=-15.0, in1=chist[:, n * ntc : (n + 1) * ntc],
                        op0=AL.add, op1=AL.add,
                    )
                    nc.vector.tensor_scalar(
                        out=chist[:, (n + 1) * ntc : (n + 2) * ntc], in0=y_n,
                        scalar1=15.0, scalar2=16.0, op0=AL.max, op1=AL.min,
                    )

                # ---- phase 4: wrap to ymod in [0, 15.5] ----
                nc.vector.tensor_scalar(
                    out=wrap[:, : 8 * ntc], in0=y_all[:, : 8 * ntc],
                    scalar1=15.75, scalar2=None, op0=AL.is_ge,
                )
                nc.vector.scalar_tensor_tensor(
                    out=ymod[:, : 8 * ntc], in0=wrap[:, : 8 * ntc], scalar=-16.0,
                    in1=y_all[:, : 8 * ntc], op0=AL.mult, op1=AL.add,
                )

                # ---- phases 5+6 per half-chunk: dists, outers, stores ----
                # iota from PSUM (single SBUF source); |.|/relu on ACT.
                # Chunk 0 outers on Pool (overlap chunk 1 phase 1, which is
                # single-source and so never contends for the shared pair);
                # last chunk outers on DVE alone (Pool already finished).
                # Halved dist phases let outers/stores start earlier.
                d4 = dtile[:, : 8 * ntc * 17].rearrange(
                    "p (n t k) -> p n t k", n=8, k=17
                )
                s4 = s17[:, : 8 * ntc * 17].rearrange(
                    "p (n t k) -> p n t k", n=8, k=17
                )
                ym4 = ymod[:, : 8 * ntc].rearrange("p (n t) -> p n t", n=8)
                s5 = s17[:, : 8 * ntc * 17].rearrange(
                    "p (i two t k) -> p i two t k", two=2, t=ntc, k=17
                )
                eng = nc.vector if last_ch else nc.gpsimd
                sgrp = 2 if last_ch else 4  # store group size in tiles
                th = ntc // 2
                for half in range(2):
                    tsl = slice(half * th, (half + 1) * th)
                    ymod_b = ym4[:, :, tsl][:, :, :, None].broadcast_to(
                        [P, 8, th, 17])
                    iota_b = iota17[:, None, None, :].broadcast_to(
                        [P, 8, th, 17])
                    nc.vector.tensor_tensor(
                        d4[:, :, tsl, :], ymod_b, iota_b, op=AL.subtract)
                    nc.scalar.activation(
                        d4[:, :, tsl, :], d4[:, :, tsl, :],
                        mybir.ActivationFunctionType.Abs,
                    )
                    nc.scalar.activation(
                        s4[:, :, tsl, :], d4[:, :, tsl, :],
                        mybir.ActivationFunctionType.Relu, bias=1.0, scale=-1.0,
                    )
                    # wraparound fold: dist[0] += s17[16]
                    nc.vector.tensor_add(
                        s4[:, :, tsl, 0:1], s4[:, :, tsl, 0:1],
                        s4[:, :, tsl, 16:17],
                    )
                    for t0 in range(half * th, (half + 1) * th, sgrp):
                        o4 = opool.tile([P, 4 * F], FP, tag="o4")
                        for ti in range(sgrp):
                            t = t0 + ti
                            o_vv = o4[:, ti * F : (ti + 1) * F].rearrange(
                                "p (i h l) -> p i h l", h=16, l=16
                            )
                            h_b = s5[:, :, 1, t, 0:16][:, :, :, None].broadcast_to(
                                [P, 4, 16, 16])
                            l_b = s5[:, :, 0, t, 0:16][:, :, None, :].broadcast_to(
                                [P, 4, 16, 16])
                            eng.tensor_mul(o_vv, h_b, l_b)
                        nc.scalar.dma_start(
                            dram_view(out_d, ch_base + t0, sgrp),
                            o4[:, : sgrp * F].rearrange("p (t f) -> p t f", t=sgrp),
                        )
                ch_base += ntc

    nc.finalize()
    return nc


_NC_CACHE = {}
LAST_RESULT = None


def kernel(**inputs) -> np.ndarray:
    global LAST_RESULT
    a = np.ascontiguousarray(np.asarray(inputs["a"], dtype=np.float32)).reshape(B_FULL, F)
    b = np.ascontiguousarray(np.asarray(inputs["b"], dtype=np.float32)).reshape(B_FULL, F)
    code2, iota17 = _const_tables()

    if ROWS not in _NC_CACHE:
        _NC_CACHE[ROWS] = build_nc(ROWS)
    nc = _NC_CACHE[ROWS]

    in_maps = []
    for c in range(N_CORES):
        in_maps.append({
            "a": a[c * ROWS : (c + 1) * ROWS],
            "b": b[c * ROWS : (c + 1) * ROWS],
            "code2": code2,
            "iota17": iota17,
        })
    res = run_bass_kernel_spmd(nc, in_maps, core_ids=list(range(N_CORES)))
    LAST_RESULT = res
    out = np.concatenate([r["out"] for r in res.results], axis=0)
    return out.reshape(B_FULL, 4, 256)
